# revision 1
# baseline (speedup 1.0000x reference)
"""Trainium2 Bass kernel for CIN (Compressed Interaction Network).

Problem: B=1024, F0=32, D=32, HID=[128,128,128], linear activations.
  layer k: z_k[b,d,(f,g)] = x0[b,f,d] * s_k[b,g,d];  h_k = z_k @ W_k + b_k
  s_{k+1} = h_k;  out = concat_k sum_d h_k  -> (B, 384)

Strategy (8 cores, batch-sharded 128 samples/core), bf16 compute with fp32
PSUM accumulation:
  "Orientation B" layout: features on partitions, n=(b,d) on free dim.
    xT[f, n] = x0[b,f,d]; h_kT[h, n] (matmul output layout == next layer's
    state layout: no transposes in the main chain).
  Broadcast tiles (row f of xT replicated across partitions) come straight
  from DRAM via broadcast-AP DMAs (DMA engines are otherwise idle); DVE
  builds 2048-wide bf16 z strips with 2x-mode multiplies; PE runs the GEMMs
  with pipelined bf16 LDWEIGHTS, 4 PSUM-bank accumulators per layer strip.
  Layer 2's full map is never materialized: out2 = vec(x0 @ h1^T) @ W2 +
  D*b2 via per-sample bilinear contractions with a block-diagonal masked
  moving operand (one 128x128 matmul per 4-sample tile).
"""
import sys

sys.path.insert(0, "/opt/trn_rl_repo")

import numpy as np
import ml_dtypes

import concourse.bass as bass
import concourse.tile as tile
from concourse import bacc, mybir
from concourse.bass_utils import run_bass_kernel_spmd

NCORES = 8
B, F0, D = 1024, 32, 32
H = 128
BL = B // NCORES          # samples per core
NTOT = BL * D             # 4096 n-columns per core
NJ = 512                  # GEMM n-chunk (one PSUM bank)
QW = 2048                 # z-strip width (4 chunks)
JPQ = QW // NJ            # 4 chunks per strip
NQ = NTOT // QW           # 2 strips
SPJ = NJ // D             # samples per n-chunk (16)
TS = 4                    # samples per 128-row tile in L2
NT = BL // TS             # 32 L2 tiles

f32 = mybir.dt.float32
bf16 = mybir.dt.bfloat16
nbf16 = ml_dtypes.bfloat16

_cache = {}


def _build_program():
    nc = bacc.Bacc("TRN2", target_bir_lowering=False, debug=False,
                   num_devices=NCORES)

    # ---- DRAM I/O (weights/constants pre-cast to bf16 on host) ----
    xT_d = nc.dram_tensor("xT", [F0, NTOT], bf16, kind="ExternalInput").ap()
    W0_d = nc.dram_tensor("W0", [128, F0 * F0 // 128, H], bf16, kind="ExternalInput").ap()
    W1_d = nc.dram_tensor("W1", [128, F0, H], bf16, kind="ExternalInput").ap()
    W2_d = nc.dram_tensor("W2", [128, F0, H], bf16, kind="ExternalInput").ap()
    b0_d = nc.dram_tensor("b0c", [H, 1], f32, kind="ExternalInput").ap()
    b1_d = nc.dram_tensor("b1c", [H, 1], f32, kind="ExternalInput").ap()
    b2_d = nc.dram_tensor("b2c", [H, 1], f32, kind="ExternalInput").ap()
    BLK_d = nc.dram_tensor("BLK", [128, TS], bf16, kind="ExternalInput").ap()
    idb_d = nc.dram_tensor("idb", [128, 128], bf16, kind="ExternalInput").ap()
    idf_d = nc.dram_tensor("idf", [128, 128], f32, kind="ExternalInput").ap()
    out_d = nc.dram_tensor("out", [BL, 3 * H], f32, kind="ExternalOutput").ap()

    with tile.TileContext(nc) as tc:
        with tc.tile_pool(name="const", bufs=1) as cpool, \
             tc.tile_pool(name="zpool", bufs=6) as zpool, \
             tc.tile_pool(name="xbsb", bufs=6) as xbpool, \
             tc.tile_pool(name="h1a", bufs=4) as h1apool, \
             tc.tile_pool(name="acc", bufs=4, space="PSUM") as accp, \
             tc.tile_pool(name="l2ps", bufs=4, space="PSUM") as l2p:

            # ---- load constants / weights ----
            xT = cpool.tile([F0, NTOT], bf16)
            nc.sync.dma_start(xT[:], xT_d)
            W0b = cpool.tile([128, F0 * F0 // 128, H], bf16)
            nc.gpsimd.dma_start(W0b[:], W0_d)
            W1b = cpool.tile([128, F0, H], bf16)
            nc.scalar.dma_start(W1b[:], W1_d)
            W2b = cpool.tile([128, F0, H], bf16)
            nc.scalar.dma_start(W2b[:], W2_d)
            b0c = cpool.tile([H, 1], f32)
            b1c = cpool.tile([H, 1], f32)
            b2c = cpool.tile([H, 1], f32)
            nc.sync.dma_start(b0c[:], b0_d)
            nc.sync.dma_start(b1c[:], b1_d)
            nc.sync.dma_start(b2c[:], b2_d)
            b2x = cpool.tile([H, 1], f32)
            nc.vector.tensor_scalar_mul(b2x[:], b2c[:], float(D))
            BLKs = cpool.tile([128, TS], bf16)
            nc.sync.dma_start(BLKs[:], BLK_d)
            idb = cpool.tile([128, 128], bf16)
            nc.sync.dma_start(idb[:], idb_d)
            idf = cpool.tile([128, 128], f32)
            nc.sync.dma_start(idf[:], idf_d)

            # XG[p, n] = xT[p % 32, n]
            XG = cpool.tile([128, NTOT], bf16)
            for q in range(4):
                nc.vector.tensor_copy(XG[q * F0:(q + 1) * F0, :], xT[:])

            # ---- state and output tensors ----
            h0T = cpool.tile([H, NTOT], bf16)
            h1T = cpool.tile([H, NTOT], bf16)
            out0T = cpool.tile([H, BL], f32)
            out1T = cpool.tile([H, BL], f32)
            out2s = cpool.tile([H, BL], f32)
            Pn = cpool.tile([128, NT * 128], bf16)  # [g, (t, f, s)]
            out_all = cpool.tile([BL, 3 * H], f32)

            # XD[(s,d), t, (f,s')] = x0[b_s, f, d] * (s == s')  (hoisted: no
            # h1 dependency, fills the startup DMA lull)
            XD = cpool.tile([128, NT, F0 * TS], bf16)
            for t in range(NT):
                with nc.named_scope("xdpre"):
                    xa_ps = l2p.tile([128, 128], bf16, tag="l2p",
                                     name=f"xaps{t}")
                    nc.tensor.transpose(xa_ps[:, :F0], xT[:, bass.ts(t, 128)],
                                        idb[:F0, :F0])
                    nc.vector.tensor_mul(
                        XD[:, t, :].rearrange("p (f s) -> p f s", s=TS),
                        xa_ps[:, :F0, None].to_broadcast((128, F0, TS)),
                        BLKs[:, None, :].to_broadcast((128, F0, TS)))

            # ================= main layers, per 2048-wide strip =========
            for q in range(NQ):
                qsl = bass.ts(q, QW)
                j0 = q * JPQ

                # ----- layer 0: z0[(4f x 32g), n] = xT[f,n]*xT[g,n] -----
                h0ps = [accp.tile([128, NJ], f32, tag="acc", name=f"h0ps{q}_{i}")
                        for i in range(JPQ)]
                dmaengs = (nc.sync, nc.gpsimd, nc.sync, nc.gpsimd)
                for c in range(8):
                    x4 = xbpool.tile([128, QW], bf16, tag="xb")
                    with nc.named_scope("x4dma"):
                        for a in range(4):
                            r = 4 * c + a
                            dmaengs[a].dma_start(
                                x4[32 * a:32 * a + 32, :],
                                xT_d[r:r + 1, qsl].broadcast_to([32, QW]))
                    z0 = zpool.tile([128, QW], bf16, tag="z")
                    with nc.named_scope("z0tt"):
                        nc.vector.tensor_mul(z0[:], XG[:, qsl], x4[:])
                    with nc.named_scope("l0mm"):
                        for jq in range(JPQ):
                            nc.tensor.matmul(h0ps[jq][:], W0b[:, c, :],
                                             z0[:, bass.ts(jq, NJ)],
                                             start=(c == 0), stop=(c == 7))
                with nc.named_scope("h0cp"):
                    for jq in range(JPQ):
                        nc.scalar.activation(
                            h0T[:, bass.ts(j0 + jq, NJ)], h0ps[jq][:],
                            mybir.ActivationFunctionType.Identity,
                            bias=b0c[:])

                # ----- layer 1: z1_f[g, n] = h0T[g,n]*xT[f,n] -----
                h1ps = [accp.tile([128, NJ], f32, tag="acc", name=f"h1ps{q}_{i}")
                        for i in range(JPQ)]
                for f in range(F0):
                    xb = xbpool.tile([128, QW], bf16, tag="xb")
                    with nc.named_scope("xbdma"):
                        dmaengs[f % 4].dma_start(
                            xb[:], xT_d[f:f + 1, qsl].broadcast_to([128, QW]))
                    z1 = zpool.tile([128, QW], bf16, tag="z")
                    with nc.named_scope("z1tt"):
                        nc.vector.tensor_mul(z1[:], h0T[:, qsl], xb[:])
                    with nc.named_scope("l1mm"):
                        for jq in range(JPQ):
                            nc.tensor.matmul(h1ps[jq][:], W1b[:, f, :],
                                             z1[:, bass.ts(jq, NJ)],
                                             start=(f == 0), stop=(f == F0 - 1))
                with nc.named_scope("h1cp"):
                    for jq in range(JPQ):
                        nc.scalar.activation(
                            h1T[:, bass.ts(j0 + jq, NJ)], h1ps[jq][:],
                            mybir.ActivationFunctionType.Identity,
                            bias=b1c[:])

                # ----- d-reductions for out0 / out1 -----
                with nc.named_scope("red"):
                    bsl = bass.ds(q * QW // D, QW // D)
                    nc.vector.reduce_sum(
                        out0T[:, bsl],
                        h0T[:, qsl].rearrange("p (b d) -> p b d", d=D),
                        axis=mybir.AxisListType.X)
                    nc.vector.reduce_sum(
                        out1T[:, bsl],
                        h1T[:, qsl].rearrange("p (b d) -> p b d", d=D),
                        axis=mybir.AxisListType.X)

                # ---- layer 2 tiles for this strip (h1T slice ready) ----
                for t in range(q * NT // NQ, (q + 1) * NT // NQ):
                    with nc.named_scope("l2"):
                        tsl = bass.ts(t, 128)
                        h1a_ps = l2p.tile([128, 128], bf16, tag="l2p")
                        nc.tensor.transpose(h1a_ps[:], h1T[:, tsl], idb[:])
                        h1a = h1apool.tile([128, 128], bf16)
                        nc.scalar.activation(h1a[:], h1a_ps[:],
                                             mybir.ActivationFunctionType.Copy)
                        # Pn_t[g, (f, s)] = sum_d h1[b_s, g, d] * x0[b_s, f, d]
                        pn_ps = l2p.tile([128, 128], f32, tag="l2p")
                        nc.tensor.matmul(pn_ps[:], h1a[:], XD[:, t, :],
                                         start=True, stop=True)
                        nc.scalar.activation(Pn[:, tsl], pn_ps[:],
                                             mybir.ActivationFunctionType.Copy)

            out2ps = l2p.tile([128, BL], f32, tag="l2p")
            PnV = Pn[:].rearrange("p (t f s) -> p t f s", f=F0, s=TS)
            with nc.named_scope("l2out"):
                for f in range(F0):
                    nc.tensor.matmul(out2ps[:], W2b[:, f, :], PnV[:, :, f, :],
                                     start=(f == 0), stop=(f == F0 - 1))
            nc.vector.tensor_scalar_add(out2s[:], out2ps[:], b2x[:])

            # ============ transpose outputs to (b, h) and store =========
            with nc.named_scope("outtp"):
                for k, src in enumerate((out0T, out1T, out2s)):
                    ops_ = l2p.tile([128, 128], f32, tag="l2p")
                    nc.tensor.transpose(ops_[:], src[:], idf[:])
                    nc.scalar.activation(out_all[:, bass.ts(k, H)], ops_[:],
                                         mybir.ActivationFunctionType.Copy)
            nc.sync.dma_start(out_d, out_all[:])

    nc.compile()
    return nc


def _consts():
    BLK = np.zeros((128, TS), nbf16)
    for p in range(128):
        BLK[p, p // 32] = 1.0
    idb = np.eye(128, dtype=nbf16)
    idf = np.eye(128, dtype=np.float32)
    return BLK, idb, idf


def kernel(inputs, W0, W1, W2, b0, b1, b2, field_size, embedding_size):
    x0 = np.ascontiguousarray(np.asarray(inputs, np.float32).reshape(B, F0, D))
    W0 = np.ascontiguousarray(np.asarray(W0, np.float32).astype(nbf16)
                              .reshape(-1, 128, H).transpose(1, 0, 2))
    W1 = np.ascontiguousarray(np.asarray(W1, np.float32).astype(nbf16)
                              .reshape(-1, 128, H).transpose(1, 0, 2))
    W2 = np.ascontiguousarray(np.asarray(W2, np.float32).astype(nbf16)
                              .reshape(-1, 128, H).transpose(1, 0, 2))
    b0 = np.asarray(b0, np.float32).reshape(H, 1)
    b1 = np.asarray(b1, np.float32).reshape(H, 1)
    b2 = np.asarray(b2, np.float32).reshape(H, 1)

    if "nc" not in _cache:
        _cache["nc"] = _build_program()
    nc = _cache["nc"]

    BLK, idb, idf = _consts()
    in_maps = []
    for c in range(NCORES):
        xs = x0[c * BL:(c + 1) * BL]                      # (128, 32, 32)
        xT = np.ascontiguousarray(
            xs.transpose(1, 0, 2).reshape(F0, NTOT)).astype(nbf16)
        in_maps.append({
            "xT": xT, "W0": W0, "W1": W1, "W2": W2,
            "b0c": b0, "b1c": b1, "b2c": b2,
            "BLK": BLK.copy(), "idb": idb.copy(), "idf": idf.copy(),
        })

    res = run_bass_kernel_spmd(nc, in_maps, list(range(NCORES)),
                               **_cache.get("run_kwargs", {}))
    _cache["last_result"] = res
    out = np.concatenate([res.results[c]["out"] for c in range(NCORES)], axis=0)
    return out.astype(np.float32)

